# revision 1
# baseline (speedup 1.0000x reference)
"""ChebConv layer (K=3) on 8 TRN2 NeuronCores, data-parallel over batch.

Math:  out = relu(sum_k T_k(L) @ x @ Theta_k),  L = 2A/lambda - I,
       T_0=I, T_1=L, T_2=2L^2-I.
Re-expanded in powers of S = (2/lambda)*A (so no identity terms on device):
       out = relu(Z_A + S @ (Z_B + S @ Z_C))
       Z_C = x@(2*Th2), Z_B = x@(Th1 - 4*Th2), Z_A = x@(Th0 - Th1 + Th2)

Host prep per core (4 batches each):
  st : [4, 1024, 1024] fp8e4m3 = 4096 * S^T per batch (scaled into fp8 range;
                                 the 1/4096 is folded into the PSUM combines)
  xt : [4, 128, 6144]  bf16    = x^T, t-pairs stacked on partitions:
                                 xt[b, (t%2)*64+f, (t//2)*1024+n] = x[b,t,n,f]
  th : [128, 384]      bf16    = [BD(2*Th2) | BD(Th1-4*Th2) | BD(Th0-Th1+Th2)]
                                 BD(M) = blockdiag(M, M) (two t's per matmul)
  out: [4, 8, 128, 768] f32    = per (batch, node-chunk): cols (h, j, t-par, o)

Hop 1 runs as fp8e4m3 DoubleRow matmuls (256-deep contraction per
instruction -> half the matmul count); only Z_C is stored in fp8 -- its
quantization error passes through BOTH strongly-contractive S-aggregations.
Hop 2 stays bf16 (its inputs' errors reach the output through only one
contraction), as do Z_B/Z_A/U.
Emission is software-pipelined (transform of step i+1 before hop1 of step i)
so the PE never waits on PSUM evacuation and stays HAM-warm.
"""

import os
import sys

import numpy as np

sys.path.insert(0, "/opt/trn_rl_repo")

B, T, N, FIN = 32, 12, 1024, 64
K, OUT_F = 3, 64
NCORES = 8
BPC = B // NCORES          # batches per core
NCHUNK = N // 128          # 8 node chunks
TPAIRS = T // 2            # 6
HALVES = 2                 # t-halves; 3 t-pairs each
JW = 3                     # t-pairs per half
SSCALE = 4096.0            # host pre-scale of S into fp8e4m3 normal range

_CACHE = {}
LAST_RESULT = None


def _build_nc():
    import concourse.bacc as bacc
    import concourse.mybir as mybir
    import concourse.tile as tile
    from contextlib import ExitStack

    dt = mybir.dt
    f32, bf16, fp8 = dt.float32, dt.bfloat16, dt.float8e4
    DR = mybir.MatmulPerfMode.DoubleRow

    nc = bacc.Bacc()
    st_d = nc.declare_dram_parameter("st", [BPC, N, N], fp8, isOutput=False)
    stb_d = nc.declare_dram_parameter("stb", [BPC, N, N], bf16, isOutput=False)
    xt_d = nc.declare_dram_parameter("xt", [BPC, 128, TPAIRS * N], bf16, isOutput=False)
    th_d = nc.declare_dram_parameter("th", [128, 3 * 128], bf16, isOutput=False)
    out_d = nc.declare_dram_parameter(
        "out", [BPC, NCHUNK, 128, HALVES * JW * 128], f32, isOutput=True
    )

    with tile.TileContext(nc) as tc, ExitStack() as ctx:
        st_pool = ctx.enter_context(tc.tile_pool(name="stp", bufs=2))
        stb_pool = ctx.enter_context(tc.tile_pool(name="stbp", bufs=2))
        xt_pool = ctx.enter_context(tc.tile_pool(name="xtp", bufs=3))
        th_pool = ctx.enter_context(tc.tile_pool(name="thp", bufs=1))
        zc_pool = ctx.enter_context(tc.tile_pool(name="zcp", bufs=3))
        zba_pool = ctx.enter_context(tc.tile_pool(name="zbap", bufs=3))
        u_pool = ctx.enter_context(tc.tile_pool(name="up", bufs=3))
        o_pool = ctx.enter_context(tc.tile_pool(name="op", bufs=3))
        ps_pool = ctx.enter_context(tc.tile_pool(name="psp", bufs=8, space="PSUM"))

        th_t = th_pool.tile([128, 3 * 128], bf16, name="th_t")
        nc.sync.dma_start(out=th_t[:], in_=th_d[:])

        st_tiles, stb_tiles, xt_tiles, zc_tiles, zba_tiles, u_tiles = {}, {}, {}, {}, {}, {}

        def emit_loads(b):
            xt_t = xt_pool.tile([128, TPAIRS * N], bf16, name=f"xt_{b}", tag="xt")
            nc.sync.dma_start(out=xt_t[:], in_=xt_d[b])
            st_t = st_pool.tile([128, NCHUNK * N], fp8, name=f"st_{b}", tag="st")
            st3 = st_t.rearrange("p (k n) -> p k n", n=N)
            sd3 = st_d[b].rearrange("(k p) n -> p k n", p=128)
            for k in range(0, NCHUNK, 2):
                nc.sync.dma_start(out=st3[:, k : k + 2], in_=sd3[:, k : k + 2])
            stb_t = stb_pool.tile([128, NCHUNK * N], bf16, name=f"stb_{b}", tag="stb")
            stb3 = stb_t.rearrange("p (k n) -> p k n", n=N)
            sb3 = stb_d[b].rearrange("(k p) n -> p k n", p=128)
            for k in range(0, NCHUNK, 2):
                nc.sync.dma_start(out=stb3[:, k : k + 2], in_=sb3[:, k : k + 2])
            st_tiles[b], stb_tiles[b], xt_tiles[b] = st_t, stb_t, xt_t

        def emit_T(b, h):
            if b not in st_tiles:
                emit_loads(b)
            xt_t = xt_tiles[b]
            # zc: per chunk c: cols [c*384, (c+1)*384) = Z_C (j-major), fp8
            # zba: per chunk c: cols [c*768, c*768+384) = Z_B (j-major),
            #                   cols [c*768+384, (c+1)*768) = Z_A (j-major), bf16
            zc = zc_pool.tile(
                [128, NCHUNK * 384], fp8, name=f"zc_{b}_{h}", tag="zc"
            )
            zba = zba_pool.tile(
                [128, NCHUNK * 2 * 384], bf16, name=f"zba_{b}_{h}", tag="zba"
            )
            zc_tiles[(b, h)], zba_tiles[(b, h)] = zc, zba
            for c in range(NCHUNK):
                for j in range(JW):
                    tp = h * JW + j
                    psT = ps_pool.tile([128, 384], f32, name=f"psT_{c}_{j}", tag="ps")
                    nc.tensor.matmul(
                        psT[:],
                        xt_t[:, tp * N + c * 128 : tp * N + (c + 1) * 128],
                        th_t[:],
                        start=True,
                        stop=True,
                    )
                    # psT cols [0:128)=Z_C, [128:256)=Z_B, [256:384)=Z_A
                    nc.vector.tensor_copy(
                        zc[:, c * 384 + j * 128 : c * 384 + (j + 1) * 128],
                        psT[:, 0:128],
                    )
                    ba_dst = zba[
                        :, c * 768 + j * 128 : c * 768 + 384 + (j + 1) * 128
                    ].rearrange("p (s x) -> p s x", x=128)[:, 0::3]
                    nc.scalar.activation(
                        ba_dst,
                        psT[:, 128:384].rearrange("p (s x) -> p s x", x=128),
                        mybir.ActivationFunctionType.Copy,
                    )

        def h1_group(b, h, c):
            st_t, zc_t, zba = st_tiles[b], zc_tiles[(b, h)], zba_tiles[(b, h)]
            st3 = st_t.rearrange("p (k n) -> p k n", n=N)
            zc3 = zc_t.rearrange("p (k r) -> p k r", r=384)
            if c == 0:
                u_tiles[(b, h)] = u_pool.tile(
                    [128, NCHUNK * 384], bf16, name=f"u_{b}_{h}", tag="u"
                )
            u_t = u_tiles[(b, h)]
            ps1 = ps_pool.tile([128, 384], f32, name=f"ps1_{c}", tag="ps")
            for q in range(NCHUNK // 2):
                nc.tensor.matmul(
                    ps1[:],
                    st3[:, 2 * q : 2 * q + 2, c * 128 : (c + 1) * 128],
                    zc3[:, 2 * q : 2 * q + 2, :],
                    start=(q == 0),
                    stop=(q == NCHUNK // 2 - 1),
                    perf_mode=DR,
                )
            nc.vector.scalar_tensor_tensor(
                u_t[:, c * 384 : (c + 1) * 384],
                ps1[:],
                1.0 / SSCALE,
                zba[:, c * 768 : c * 768 + 384],
                op0=mybir.AluOpType.mult,
                op1=mybir.AluOpType.add,
            )

        o_tiles = {}

        def h2_group(b, h, c):
            stb_t, zba, u_t = stb_tiles[b], zba_tiles[(b, h)], u_tiles[(b, h)]
            if c == 0:
                o_tiles[(b, h)] = o_pool.tile(
                    [128, NCHUNK * 384], f32, name=f"o_{b}_{h}", tag="o"
                )
            o_t = o_tiles[(b, h)]
            ps2 = ps_pool.tile([128, 384], f32, name=f"ps2_{c}", tag="ps")
            for k in range(NCHUNK):
                nc.tensor.matmul(
                    ps2[:],
                    stb_t[:, k * N + c * 128 : k * N + (c + 1) * 128],
                    u_t[:, k * 384 : (k + 1) * 384],
                    start=(k == 0),
                    stop=(k == NCHUNK - 1),
                )
            osl = o_t[:, c * 384 : (c + 1) * 384]
            nc.vector.tensor_add(
                osl.rearrange("p (j x) -> p j x", x=128),
                ps2.rearrange("p (j x) -> p j x", x=128),
                zba[:, c * 768 + 384 : (c + 1) * 768].rearrange(
                    "p (j x) -> p j x", x=128
                ),
            )
            nc.scalar.activation(osl, osl, mybir.ActivationFunctionType.Relu)
            nc.sync.dma_start(
                out=out_d[b, c, :, h * 384 : (h + 1) * 384], in_=osl
            )

        # Three-stage skewed pipeline: block i emits T(i+1), then H1(i) groups
        # interleaved 1:1 with H2(i-1) groups so bf16 matmuls keep the PE
        # HAM-warm through the DoubleRow stretches.
        steps = [(b, h) for b in range(BPC) for h in range(HALVES)]
        emit_T(*steps[0])
        emit_T(*steps[1])
        for c in range(NCHUNK):
            h1_group(*steps[0], c)
        for i in range(1, len(steps)):
            if i + 1 < len(steps):
                emit_T(*steps[i + 1])
            for c in range(NCHUNK):
                h1_group(*steps[i], c)
                h2_group(*steps[i - 1], c)
        for c in range(NCHUNK):
            h2_group(*steps[-1], c)
    nc.compile()
    return nc


def _get_nc():
    if "nc" not in _CACHE:
        _CACHE["nc"] = _build_nc()
    return _CACHE["nc"]


def _prep_core(x_c, A_c, TH):
    import ml_dtypes

    lam = np.maximum(A_c.sum(axis=-1).max(axis=-1), 1.0)  # [BPC]
    sT = A_c.transpose(0, 2, 1) * (2.0 / lam)[:, None, None]
    st = np.ascontiguousarray(
        np.clip(sT * SSCALE, 0.0, 240.0).astype(ml_dtypes.float8_e4m3)
    )
    stb = np.ascontiguousarray(sT.astype(ml_dtypes.bfloat16))
    xt = np.ascontiguousarray(
        x_c.reshape(BPC, TPAIRS, 2, N, FIN)
        .transpose(0, 2, 4, 1, 3)
        .reshape(BPC, 128, TPAIRS * N)
        .astype(ml_dtypes.bfloat16)
    )
    return {"st": st, "stb": stb, "xt": xt, "th": TH}


def kernel(x, A, Theta):
    global LAST_RESULT
    import ml_dtypes
    from concourse.bass_utils import run_bass_kernel_spmd

    x = np.asarray(x, dtype=np.float32)
    A = np.asarray(A, dtype=np.float32)
    Theta = np.asarray(Theta, dtype=np.float32)

    T0, T1, T2 = Theta[0], Theta[1], Theta[2]
    folded = [2.0 * T2, T1 - 4.0 * T2, T0 - T1 + T2]
    TH = np.zeros((128, 3 * 128), np.float32)
    for q, M in enumerate(folded):
        TH[0:64, q * 128 : q * 128 + 64] = M
        TH[64:128, q * 128 + 64 : q * 128 + 128] = M
    TH = TH.astype(ml_dtypes.bfloat16)

    nc = _get_nc()
    in_maps = [
        _prep_core(x[c * BPC : (c + 1) * BPC], A[c * BPC : (c + 1) * BPC], TH)
        for c in range(NCORES)
    ]
    trace = bool(int(os.environ.get("CHEB_TRACE", "0")))
    res = run_bass_kernel_spmd(nc, in_maps, list(range(NCORES)), trace=trace)
    LAST_RESULT = res

    outs = []
    for c in range(NCORES):
        od = np.asarray(res.results[c]["out"])  # [BPC, 8, 128, 768]
        r = (
            od.reshape(BPC, NCHUNK, 128, HALVES, JW, 2, OUT_F)
            .transpose(0, 3, 4, 5, 1, 2, 6)
            .reshape(BPC, T, N, OUT_F)
        )
        outs.append(r)
    return np.ascontiguousarray(np.concatenate(outs, axis=0).astype(np.float32))



# revision 4
# speedup vs baseline: 1.0247x; 1.0247x over previous
"""ChebConv layer (K=3) on 8 TRN2 NeuronCores, data-parallel over batch.

Math:  out = relu(sum_k T_k(L) @ x @ Theta_k),  L = 2A/lambda - I,
       T_0=I, T_1=L, T_2=2L^2-I.
Re-expanded in powers of S = (2/lambda)*A (so no identity terms on device):
       out = relu(Z_A + S @ (Z_B + S @ Z_C))
       Z_C = x@(2*Th2), Z_B = x@(Th1 - 4*Th2), Z_A = x@(Th0 - Th1 + Th2)

Host prep per core (4 batches each):
  st : [4, 1024, 1024] fp8e4m3 = 4096 * S^T per batch (scaled into fp8 range;
                                 the 1/4096 is folded into the PSUM evacs)
  xt : [4, 128, 6144]  bf16    = x^T, t-pairs stacked on partitions:
                                 xt[b, (t%2)*64+f, (t//2)*1024+n] = x[b,t,n,f]
  th : [128, 384]      bf16    = [BD(2*Th2) | BD(Th1-4*Th2) |
                                  4096*BD(Th0-Th1+Th2)]
                                 BD(M) = blockdiag(M, M) (two t's per matmul)
  out: [4, 8, 128, 768] bf16   = per (batch, node-chunk): cols (h, j, t-par, o)

The PE is PSUM-column-write bound (~163ns per 384-wide matmul regardless of
dtype), so the win is fewer matmuls: BOTH hops run as fp8e4m3 DoubleRow
matmuls (256-deep contraction -> 4 instead of 8 instructions per chunk).
u = Z_B + S@Z_C is quantized to fp8 for hop 2; its error passes through one
strongly-contractive S-aggregation and lands on a term ~20x smaller than the
dominant Z_A, keeping rel err ~1e-2 under the 2e-2 gate.

Transform PSUM is one 3-bank tile per chunk ([128, 3, 512] f32) so each
evacuation is a single wide instruction. Evac work is spread over three
engines: DVE casts Z_C->fp8 and does the u evac (scale+add+fp8), Act copies
the B|A staging and the final relu (with the 1/4096 descale fused), GpSimd
does the Z_A add into hop-2 PSUM. Emission is software-pipelined per chunk
slot: h1(i,c), h2(i-1,c), T(i+1,c), so the PE never waits on evacuation.
"""

import os
import sys

import numpy as np

sys.path.insert(0, "/opt/trn_rl_repo")

B, T, N, FIN = 32, 12, 1024, 64
K, OUT_F = 3, 64
NCORES = 8
BPC = B // NCORES          # batches per core
NCHUNK = N // 128          # 8 node chunks
TPAIRS = T // 2            # 6
HALVES = 2                 # t-halves; 3 t-pairs each
JW = 3                     # t-pairs per half
SSCALE = 4096.0            # host pre-scale of S into fp8e4m3 normal range

_CACHE = {}
LAST_RESULT = None


def _build_nc():
    import concourse.bacc as bacc
    import concourse.mybir as mybir
    import concourse.tile as tile
    from contextlib import ExitStack

    dt = mybir.dt
    f32, bf16, fp8 = dt.float32, dt.bfloat16, dt.float8e4
    DR = mybir.MatmulPerfMode.DoubleRow

    nc = bacc.Bacc()
    st_d = nc.declare_dram_parameter("st", [BPC, N, N], fp8, isOutput=False)
    xt_d = nc.declare_dram_parameter("xt", [BPC, 128, TPAIRS * N], bf16, isOutput=False)
    th_d = nc.declare_dram_parameter("th", [128, 3 * 128], bf16, isOutput=False)
    out_d = nc.declare_dram_parameter(
        "out", [BPC, NCHUNK, 128, HALVES * JW * 128], bf16, isOutput=True
    )

    nsteps = BPC * HALVES
    steps = [(b, h) for b in range(BPC) for h in range(HALVES)]

    with tile.TileContext(nc) as tc, ExitStack() as ctx:
        st_pool = ctx.enter_context(tc.tile_pool(name="stp", bufs=2))
        xt_pool = ctx.enter_context(tc.tile_pool(name="xtp", bufs=2))
        th_pool = ctx.enter_context(tc.tile_pool(name="thp", bufs=1))
        zc_pool = ctx.enter_context(tc.tile_pool(name="zcp", bufs=3))
        zba_pool = ctx.enter_context(tc.tile_pool(name="zbap", bufs=3))
        u_pool = ctx.enter_context(tc.tile_pool(name="up", bufs=3))
        o_pool = ctx.enter_context(tc.tile_pool(name="op", bufs=3))
        pst_pool = ctx.enter_context(tc.tile_pool(name="pstp", bufs=2, space="PSUM"))
        ps1_pool = ctx.enter_context(tc.tile_pool(name="ps1p", bufs=1, space="PSUM"))
        ps2_pool = ctx.enter_context(tc.tile_pool(name="ps2p", bufs=1, space="PSUM"))

        th_t = th_pool.tile([128, 3 * 128], bf16, name="th_t")
        nc.sync.dma_start(out=th_t[:], in_=th_d[:])

        st_tiles, xt_tiles = {}, {}
        zc_tiles, zba_tiles, u_tiles, o_tiles = {}, {}, {}, {}

        def emit_loads(b):
            xt_t = xt_pool.tile([128, TPAIRS * N], bf16, name=f"xt_{b}", tag="xt")
            nc.sync.dma_start(out=xt_t[:], in_=xt_d[b])
            st_t = st_pool.tile([128, NCHUNK * N], fp8, name=f"st_{b}", tag="st")
            st3 = st_t.rearrange("p (k n) -> p k n", n=N)
            sd3 = st_d[b].rearrange("(k p) n -> p k n", p=128)
            for k in range(0, NCHUNK, 2):
                nc.sync.dma_start(out=st3[:, k : k + 2], in_=sd3[:, k : k + 2])
            st_tiles[b], xt_tiles[b] = st_t, xt_t

        def emit_T(i, c):
            b, h = steps[i]
            if c == 0:
                # kick loads for the next batch 2 steps ahead of first use
                if h == 1 and b + 1 < BPC and (b + 1) not in st_tiles:
                    emit_loads(b + 1)
                zc_tiles[i] = zc_pool.tile(
                    [128, NCHUNK * 384], fp8, name=f"zc_{i}", tag="zc"
                )
                zba_tiles[i] = zba_pool.tile(
                    [128, NCHUNK * 768], bf16, name=f"zba_{i}", tag="zba"
                )
            xt_t = xt_tiles[b]
            zc, zba = zc_tiles[i], zba_tiles[i]
            psT = pst_pool.tile([128, 3, 512], f32, name=f"psT_{i}_{c}", tag="pst")
            for j in range(JW):
                tp = h * JW + j
                nc.tensor.matmul(
                    psT[:, j, 0:384],
                    xt_t[:, tp * N + c * 128 : tp * N + (c + 1) * 128],
                    th_t[:],
                    start=True,
                    stop=True,
                )
            # psT[:, j, 0:128]=Z_C_j, [128:256]=Z_B_j, [256:384]=4096*Z_A_j
            # zc casts alternate V/S to balance the evacuation load
            zc_dst = zc[:, c * 384 : (c + 1) * 384].rearrange(
                "p (j x) -> p j x", x=128
            )
            if c % 2 == 0:
                nc.vector.tensor_copy(zc_dst, psT[:, :, 0:128])
            else:
                nc.scalar.activation(
                    zc_dst, psT[:, :, 0:128], mybir.ActivationFunctionType.Copy
                )
            nc.scalar.activation(
                zba[:, c * 768 : (c + 1) * 768].rearrange("p (j x) -> p j x", x=256),
                psT[:, :, 128:384],
                mybir.ActivationFunctionType.Copy,
            )

        def h1_group(i, c):
            b, h = steps[i]
            if c == 0:
                u_tiles[i] = u_pool.tile(
                    [128, NCHUNK * 384], fp8, name=f"u_{i}", tag="u"
                )
            st3 = st_tiles[b].rearrange("p (k n) -> p k n", n=N)
            zc3 = zc_tiles[i].rearrange("p (k r) -> p k r", r=384)
            # zba per chunk: [B0 A0 B1 A1 B2 A2] blocks of 128
            zb = zba_tiles[i][:, c * 768 : (c + 1) * 768].rearrange(
                "p (j s x) -> p j s x", s=2, x=128
            )[:, :, 0]
            ps1 = ps1_pool.tile([128, 384], f32, name=f"ps1_{c}", tag="ps1")
            for q in range(NCHUNK // 2):
                nc.tensor.matmul(
                    ps1[:],
                    st3[:, 2 * q : 2 * q + 2, c * 128 : (c + 1) * 128],
                    zc3[:, 2 * q : 2 * q + 2, :],
                    start=(q == 0),
                    stop=(q == NCHUNK // 2 - 1),
                    perf_mode=DR,
                )
            nc.vector.scalar_tensor_tensor(
                u_tiles[i][:, c * 384 : (c + 1) * 384].rearrange(
                    "p (j x) -> p j x", x=128
                ),
                ps1.rearrange("p (j x) -> p j x", x=128),
                1.0 / SSCALE,
                zb,
                op0=mybir.AluOpType.mult,
                op1=mybir.AluOpType.add,
            )

        def h2_group(i, c):
            b, h = steps[i]
            if c == 0:
                o_tiles[i] = o_pool.tile(
                    [128, NCHUNK * 384], bf16, name=f"o_{i}", tag="o"
                )
            st3 = st_tiles[b].rearrange("p (k n) -> p k n", n=N)
            u3 = u_tiles[i].rearrange("p (k r) -> p k r", r=384)
            za = zba_tiles[i][:, c * 768 : (c + 1) * 768].rearrange(
                "p (j s x) -> p j s x", s=2, x=128
            )[:, :, 1]
            ps2 = ps2_pool.tile([128, 384], f32, name=f"ps2_{c}", tag="ps2")
            for q in range(NCHUNK // 2):
                nc.tensor.matmul(
                    ps2[:],
                    st3[:, 2 * q : 2 * q + 2, c * 128 : (c + 1) * 128],
                    u3[:, 2 * q : 2 * q + 2, :],
                    start=(q == 0),
                    stop=(q == NCHUNK // 2 - 1),
                    perf_mode=DR,
                )
            # ps2 += 4096*Z_A (in place; GPSIMD can't touch PSUM, so DVE),
            # then out = relu(ps2/4096) in bf16
            nc.vector.tensor_tensor(
                ps2.rearrange("p (j x) -> p j x", x=128),
                ps2.rearrange("p (j x) -> p j x", x=128),
                za,
                op=mybir.AluOpType.add,
            )
            osl = o_tiles[i][:, c * 384 : (c + 1) * 384]
            nc.scalar.activation(
                osl,
                ps2[:],
                mybir.ActivationFunctionType.Relu,
                scale=1.0 / SSCALE,
            )
            nc.sync.dma_start(out=out_d[b, c, :, h * 384 : (h + 1) * 384], in_=osl)

        # Software pipeline: chunk slot c of step i runs h1(i,c), h2(i-1,c),
        # T(i+1,c) back-to-back on the PE so evacuations (V/S/G) overlap the
        # next group's matmuls.
        emit_loads(0)
        for c in range(NCHUNK):
            emit_T(0, c)
        for i in range(nsteps):
            for c in range(NCHUNK):
                h1_group(i, c)
                if i > 0:
                    h2_group(i - 1, c)
                if i + 1 < nsteps:
                    emit_T(i + 1, c)
        for c in range(NCHUNK):
            h2_group(nsteps - 1, c)
    nc.compile()
    return nc


def _get_nc():
    if "nc" not in _CACHE:
        _CACHE["nc"] = _build_nc()
    return _CACHE["nc"]


def _prep_core(x_c, A_c, TH):
    import ml_dtypes

    lam = np.maximum(A_c.sum(axis=-1).max(axis=-1), 1.0)  # [BPC]
    sT = A_c.transpose(0, 2, 1) * (2.0 / lam)[:, None, None]
    st = np.ascontiguousarray(
        np.clip(sT * SSCALE, 0.0, 240.0).astype(ml_dtypes.float8_e4m3)
    )
    xt = np.ascontiguousarray(
        x_c.reshape(BPC, TPAIRS, 2, N, FIN)
        .transpose(0, 2, 4, 1, 3)
        .reshape(BPC, 128, TPAIRS * N)
        .astype(ml_dtypes.bfloat16)
    )
    return {"st": st, "xt": xt, "th": TH}


def kernel(x, A, Theta):
    global LAST_RESULT
    import ml_dtypes
    from concourse.bass_utils import run_bass_kernel_spmd

    x = np.asarray(x, dtype=np.float32)
    A = np.asarray(A, dtype=np.float32)
    Theta = np.asarray(Theta, dtype=np.float32)

    T0, T1, T2 = Theta[0], Theta[1], Theta[2]
    folded = [2.0 * T2, T1 - 4.0 * T2, SSCALE * (T0 - T1 + T2)]
    TH = np.zeros((128, 3 * 128), np.float32)
    for q, M in enumerate(folded):
        TH[0:64, q * 128 : q * 128 + 64] = M
        TH[64:128, q * 128 + 64 : q * 128 + 128] = M
    TH = TH.astype(ml_dtypes.bfloat16)

    nc = _get_nc()
    in_maps = [
        _prep_core(x[c * BPC : (c + 1) * BPC], A[c * BPC : (c + 1) * BPC], TH)
        for c in range(NCORES)
    ]
    trace = bool(int(os.environ.get("CHEB_TRACE", "0")))
    res = run_bass_kernel_spmd(nc, in_maps, list(range(NCORES)), trace=trace)
    LAST_RESULT = res

    outs = []
    for c in range(NCORES):
        od = np.asarray(res.results[c]["out"]).astype(np.float32)
        r = (
            od.reshape(BPC, NCHUNK, 128, HALVES, JW, 2, OUT_F)
            .transpose(0, 3, 4, 5, 1, 2, 6)
            .reshape(BPC, T, N, OUT_F)
        )
        outs.append(r)
    return np.ascontiguousarray(np.concatenate(outs, axis=0).astype(np.float32))
